# revision 25
# baseline (speedup 1.0000x reference)
"""Cost-volume kernel for Trainium2 (8 NeuronCores, batch-parallel). v2

Problem: cost[b, o=(dy,dx), h, w] = PReLU(mean_c(c1[b,c,h,w] *
         pad(warped)[b,c,h+dy,w+dx]), alpha), 81 offsets (9x9), zero pad 4.

Strategy per core (one batch element per NeuronCore):
  - Image tiled 16x8 pixels, M=128 pixels per matmul tile.  Pixel->partition
    map is cluster-major: p = (a//4)*32 + b8*4 + (a%4)  (a=row-in-tile,
    b8=col-in-tile).  TensorE computes the gram of the pixel tile against its
    24x16 halo: PSUM[p, n] = sum_c c1[c, p] * wpad[c, halo_n], n = hh*16+ww,
    K = 96+96 chunks, bf16 in / fp32 acc.
  - The 81 cost entries of pixel (a, b8) live at n = (a+dy)*16 + (b8+dx).
    For a cluster c (a in [4c, 4c+4)) the union of all windows is the
    partition-uniform range [c*64, c*64+192) -- and clusters are 32
    CONSECUTIVE partitions, so a single engine copy per cluster compacts
    the gram into a per-partition-contiguous window buffer.  The out-DMA
    is then 1 descriptor per partition (7.7KB) instead of 288B shreds.
  - Host finishes with a cheap numpy diagonal gather (192 -> 81 per pixel,
    j = (a%4 + dy)*16 + b8 + dx), PReLU and the 1/192 mean scale.
"""

import numpy as np

B, C, H, W = 8, 192, 128, 160
R = 4
TH, TW = 16, 8                    # pixel tile
HH, HWW = TH + 2 * R, TW + 2 * R  # halo 24 x 16
NCOL = HH * HWW                   # 384 matmul free dim
BANDS = H // TH                   # 8 row bands
TPB = W // TW                     # 20 tiles per band
PH, PW = H + 2 * R, W + 2 * R     # padded 136 x 168
K0, K1 = 96, 96                   # contraction chunks
NCLUST = 4                        # a-clusters per tile (4 rows each)
WIN = (4 + 8) * HWW               # 192: 12 halo rows x 16 cols per cluster
# interior row sections: fine-grained at the top so band 0 starts early
SECS = ((0, 8), (8, 16), (16, 24), (24, 32), (32, 48), (48, 64),
        (64, 80), (80, 96), (96, 112), (112, 128))

_CACHE = {}


def _build():
    if "nc" in _CACHE:
        return _CACHE["nc"]
    import sys
    if "/opt/trn_rl_repo" not in sys.path:
        sys.path.insert(0, "/opt/trn_rl_repo")
    import concourse.mybir as mybir
    import concourse.tile as tile
    from concourse import bacc
    from concourse.bass import AP

    nc = bacc.Bacc(None, target_bir_lowering=False)
    bf16 = mybir.dt.bfloat16
    f32 = mybir.dt.float32

    # c1 pre-tiled on host: [C, band, t, m], m = (a//4)*32 + b8*4 + a%4
    c1_d = nc.dram_tensor("c1b", [C, H * W], bf16, kind="ExternalInput")
    wp_d = nc.dram_tensor("wpad", [C, H * PW], bf16, kind="ExternalInput")
    go_d = nc.dram_tensor("gout", [BANDS * 128, TPB * WIN], bf16,
                          kind="ExternalOutput")

    with tile.TileContext(nc) as tc:
        with (
            tc.tile_pool(name="wp", bufs=1) as wp_pool,
            tc.tile_pool(name="c1", bufs=3) as c1_pool,
            tc.tile_pool(name="st", bufs=2) as st_pool,
            tc.tile_pool(name="wn", bufs=2) as wn_pool,
            tc.tile_pool(name="ps", bufs=4, space="PSUM") as ps_pool,
        ):
            # persistent padded warped, full-resident, one tile per k-chunk.
            # Host sends only the col-padded interior [C, H, PW]; the 4-row
            # top/bottom zero borders are memset on device, and the interior
            # streams in NSEC 16-row sections so band 0 can start early.
            wp_sb = []
            for k, (ks, kn) in enumerate(((0, K0), (K0, K1))):
                t = wp_pool.tile([kn, PH * PW], bf16, tag=f"wp{k}")
                nc.gpsimd.memset(t[:, 0:R * PW], 0.0)
                nc.gpsimd.memset(t[:, (R + H) * PW:PH * PW], 0.0)
                wp_sb.append(t)
            for r0_, r1_ in SECS:
                for k, (ks, kn) in enumerate(((0, K0), (K0, K1))):
                    nc.sync.dma_start(
                        wp_sb[k][:, (R + r0_) * PW:(R + r1_) * PW],
                        wp_d[ks:ks + kn, r0_ * PW:r1_ * PW])

            def load_c1(band, t0=0, t1=TPB, tiles=None):
                # c1 band loads on the idle GPSIMD HWDGE queue so their
                # issue never waits behind copy work on ACT/DVE
                if tiles is None:
                    c1t_a = c1_pool.tile([K0, TPB * 128], bf16, tag="c1_0")
                    c1t_b = c1_pool.tile([K1, TPB * 128], bf16, tag="c1_1")
                    tiles = [c1t_a, c1t_b]
                for k, (ks, kn) in enumerate(((0, K0), (K0, K1))):
                    nc.gpsimd.dma_start(
                        tiles[k][:, t0 * 128:t1 * 128],
                        c1_d[ks:ks + kn,
                             (band * TPB + t0) * 128:
                             (band * TPB + t1) * 128])
                return tiles

            # PE warm-up burst during the initial DMA window: ~5us of
            # back-to-back dummy matmuls flips the HAM clock gate to 2.4GHz
            # before the real stream starts.
            warm = c1_pool.tile([128, 512], bf16, tag="warm")
            nc.gpsimd.memset(warm[:], 0.0)
            for _ in range(16):
                ps_w = ps_pool.tile([128, 1024], f32, tag="ps")
                nc.tensor.matmul(ps_w[:, 0:512], warm[:, 0:128],
                                 warm[:, 0:512], start=True, stop=True)

            # band 0 c1 split: first half gates the first matmul; the
            # second half and band 1 stream in behind it
            c1_0 = load_c1(0, 0, TPB // 2)
            load_c1(0, TPB // 2, TPB, tiles=c1_0)
            c1_bufs = [c1_0, load_c1(1)]
            HT = TPB // 2             # tiles per half-band
            for band in range(BANDS):
                r0 = band * TH
                c1_sb = c1_bufs.pop(0)

                staged = st_pool.tile([128, TPB * NCOL], bf16, tag="staged")
                sap0 = staged[:]
                srow = sap0.ap[0][0]

                for half in range(2):
                    for tp in range(TPB // 4):
                        pi = half * (TPB // 4) + tp
                        ps = ps_pool.tile([128, 1024], f32, tag="ps")
                        for hf in range(2):
                            t_i = 2 * pi + hf
                            c0 = t_i * TW
                            for k, kn in enumerate((K0, K1)):
                                a1 = c1_sb[k][:]
                                lhsT = AP(a1.tensor,
                                          a1.offset + t_i * 128,
                                          [[a1.ap[0][0], kn], [1, 128]])
                                a2 = wp_sb[k][:]
                                rhs = AP(a2.tensor,
                                         a2.offset + r0 * PW + c0,
                                         [[a2.ap[0][0], kn],
                                          [PW, HH], [1, HWW]])
                                nc.tensor.matmul(
                                    ps[:, hf * 512:hf * 512 + NCOL],
                                    lhsT, rhs,
                                    start=(k == 0), stop=(k == 1))
                        # one copy moves both tiles' grams; spread over
                        # DVE / ACT / GPSIMD to keep each engine <50% busy
                        pap = ps[:]
                        src2 = AP(pap.tensor, pap.offset,
                                  [[pap.ap[0][0], 128], [512, 2],
                                   [1, NCOL]])
                        d0 = 2 * pi * NCOL
                        dst2 = staged[:, d0:d0 + 2 * NCOL]
                        if pi in (1, 8):
                            nc.vector.tensor_copy(dst2, src2)
                        else:
                            nc.scalar.copy(dst2, src2)
                        # deferred 2-ahead c1 prefetch
                        if pi == 0 and band + 2 < BANDS:
                            c1_bufs.append(load_c1(band + 2))

                    # compact half-band: per cluster of 32 consecutive
                    # partitions, window [c*64, c*64+192) of each 384-block.
                    # The very last half-band is split in two quarters so
                    # the final out-DMA drains a smaller tail.
                    win = wn_pool.tile([128, HT * WIN], bf16, tag="win")
                    wap = win[:]
                    wrow = wap.ap[0][0]
                    qsplits = ((0, HT),)
                    if band == BANDS - 1 and half == 1:
                        qsplits = ((0, 6), (6, HT))
                    for q0, q1 in qsplits:
                        for cc in range(NCLUST):
                            src = AP(sap0.tensor,
                                     sap0.offset + cc * 32 * srow
                                     + (half * HT + q0) * NCOL + cc * 64,
                                     [[srow, 32], [NCOL, q1 - q0],
                                      [1, WIN]])
                            dst = AP(wap.tensor,
                                     wap.offset + cc * 32 * wrow + q0 * WIN,
                                     [[wrow, 32], [WIN, q1 - q0],
                                      [1, WIN]])
                            nc.vector.tensor_copy(dst, src)
                        # out: one descriptor per partition
                        gap = go_d[:]
                        src = AP(wap.tensor, wap.offset + q0 * WIN,
                                 [[wrow, 128], [1, (q1 - q0) * WIN]])
                        dst = AP(gap.tensor,
                                 gap.offset + (2 * band + half) * 128
                                 * (HT * WIN) + q0 * WIN,
                                 [[HT * WIN, 128], [1, (q1 - q0) * WIN]])
                        nc.gpsimd.dma_start(dst, src)

    nc.finalize()
    _CACHE["nc"] = nc
    return nc


def kernel(c1, warped, alpha):
    import sys
    if "/opt/trn_rl_repo" not in sys.path:
        sys.path.insert(0, "/opt/trn_rl_repo")
    import ml_dtypes
    from concourse.bass_utils import run_bass_kernel_spmd

    nc = _build()
    bf = ml_dtypes.bfloat16

    in_maps = []
    for b in range(B):
        # col-padded interior only; device memsets the 4-row borders
        wpad = np.zeros((C, H, PW), np.float32)
        wpad[:, :, R:R + W] = warped[b]
        # tile c1: [C, band, a, t, b8] -> [C, band, t, c, b8, q]
        # column m = c*32 + b8*4 + q, where a = 4c + q
        c1t = np.asarray(c1[b]).reshape(C, BANDS, NCLUST, 4, TPB, TW)
        c1t = np.ascontiguousarray(c1t.transpose(0, 1, 4, 2, 5, 3))
        in_maps.append({
            "c1b": c1t.reshape(C, H * W).astype(bf),
            "wpad": wpad.reshape(C, H * PW).astype(bf),
        })

    import os
    trace = bool(int(os.environ.get("COSTVOL_TRACE", "0")))
    res = run_bass_kernel_spmd(nc, in_maps, core_ids=list(range(B)),
                               trace=trace)
    if trace:
        _CACHE["last_exec_time_ns"] = res.exec_time_ns

    # host-side: diagonal gather + mean + PReLU
    a_val = float(np.asarray(alpha).reshape(-1)[0])
    dy, dx = np.meshgrid(np.arange(9), np.arange(9), indexing="ij")
    dy = dy.reshape(-1)
    dx = dx.reshape(-1)                                      # [81]
    qq = np.arange(4)[:, None, None]
    bb8 = np.arange(TW)[None, :, None]
    # j = (q+dy)*16 + (b8+dx), shape [q, b8, 81]
    jidx = (qq + dy[None, None, :]) * HWW + bb8 + dx[None, None, :]

    out = np.empty((B, 81, H, W), np.float32)
    for b in range(B):
        g = np.asarray(res.results[b]["gout"]).astype(np.float32)
        # [band, half, c, b8, q, th, j]
        g = g.reshape(BANDS, 2, NCLUST, TW, 4, TPB // 2, WIN)
        got = np.take_along_axis(
            g, jidx.transpose(1, 0, 2)[None, None, None, :, :, None, :],
            axis=6)
        # got: [band, half, c, b8, q, th, 81] -> [81, band, c, q, half, th, b8]
        cost = got.transpose(6, 0, 2, 4, 1, 5, 3).reshape(81, H, W) * (1.0 / C)
        out[b] = np.where(cost >= 0, cost, a_val * cost)
    return out


# revision 27
# speedup vs baseline: 1.1637x; 1.1637x over previous
"""Cost-volume kernel for Trainium2 (8 NeuronCores, batch-parallel). v2

Problem: cost[b, o=(dy,dx), h, w] = PReLU(mean_c(c1[b,c,h,w] *
         pad(warped)[b,c,h+dy,w+dx]), alpha), 81 offsets (9x9), zero pad 4.

Strategy per core (one batch element per NeuronCore):
  - Image tiled 16x8 pixels, M=128 pixels per matmul tile.  Pixel->partition
    map is cluster-major: p = (a//4)*32 + b8*4 + (a%4)  (a=row-in-tile,
    b8=col-in-tile).  TensorE computes the gram of the pixel tile against its
    24x16 halo: PSUM[p, n] = sum_c c1[c, p] * wpad[c, halo_n], n = hh*16+ww,
    K = 96+96 chunks, bf16 in / fp32 acc.
  - The 81 cost entries of pixel (a, b8) live at n = (a+dy)*16 + (b8+dx).
    For a cluster c (a in [4c, 4c+4)) the union of all windows is the
    partition-uniform range [c*64, c*64+192) -- and clusters are 32
    CONSECUTIVE partitions, so a single engine copy per cluster compacts
    the gram into a per-partition-contiguous window buffer.  The out-DMA
    is then 1 descriptor per partition (7.7KB) instead of 288B shreds.
  - Host finishes with a cheap numpy diagonal gather (192 -> 81 per pixel,
    j = (a%4 + dy)*16 + b8 + dx), PReLU and the 1/192 mean scale.
"""

import numpy as np

B, C, H, W = 8, 192, 128, 160
R = 4
TH, TW = 16, 8                    # pixel tile
HH, HWW = TH + 2 * R, TW + 2 * R  # halo 24 x 16
NCOL = HH * HWW                   # 384 matmul free dim
BANDS = H // TH                   # 8 row bands
TPB = W // TW                     # 20 tiles per band
PH, PW = H + 2 * R, W + 2 * R     # padded 136 x 168
K0, K1 = 96, 96                   # contraction chunks
NCLUST = 4                        # a-clusters per tile (4 rows each)
WIN = (4 + 8) * HWW               # 192: 12 halo rows x 16 cols per cluster
# interior row sections: fine-grained at the top so band 0 starts early
SECS = ((0, 8), (8, 16), (16, 24), (24, 32), (32, 48), (48, 64),
        (64, 80), (80, 96), (96, 112), (112, 128))

_CACHE = {}


def _build():
    if "nc" in _CACHE:
        return _CACHE["nc"]
    import sys
    if "/opt/trn_rl_repo" not in sys.path:
        sys.path.insert(0, "/opt/trn_rl_repo")
    import concourse.mybir as mybir
    import concourse.tile as tile
    from concourse import bacc
    from concourse.bass import AP

    nc = bacc.Bacc(None, target_bir_lowering=False)
    bf16 = mybir.dt.bfloat16
    f32 = mybir.dt.float32

    # c1 pre-tiled on host: [C, band, t, m], m = (a//4)*32 + b8*4 + a%4
    c1_d = nc.dram_tensor("c1b", [C, H * W], bf16, kind="ExternalInput")
    wp_d = nc.dram_tensor("wpad", [C, H * PW], bf16, kind="ExternalInput")
    go_d = nc.dram_tensor("gout", [BANDS * 128, TPB * WIN], bf16,
                          kind="ExternalOutput")

    with tile.TileContext(nc) as tc:
        with (
            tc.tile_pool(name="wp", bufs=1) as wp_pool,
            tc.tile_pool(name="c1", bufs=3) as c1_pool,
            tc.tile_pool(name="st", bufs=2) as st_pool,
            tc.tile_pool(name="wn", bufs=12) as wn_pool,
            tc.tile_pool(name="ps", bufs=4, space="PSUM") as ps_pool,
        ):
            # persistent padded warped, full-resident, one tile per k-chunk.
            # Host sends only the col-padded interior [C, H, PW]; the 4-row
            # top/bottom zero borders are memset on device, and the interior
            # streams in NSEC 16-row sections so band 0 can start early.
            wp_sb = []
            for k, (ks, kn) in enumerate(((0, K0), (K0, K1))):
                t = wp_pool.tile([kn, PH * PW], bf16, tag=f"wp{k}")
                nc.gpsimd.memset(t[:, 0:R * PW], 0.0)
                nc.gpsimd.memset(t[:, (R + H) * PW:PH * PW], 0.0)
                wp_sb.append(t)
            for r0_, r1_ in SECS:
                for k, (ks, kn) in enumerate(((0, K0), (K0, K1))):
                    nc.sync.dma_start(
                        wp_sb[k][:, (R + r0_) * PW:(R + r1_) * PW],
                        wp_d[ks:ks + kn, r0_ * PW:r1_ * PW])

            def load_c1(band, t0=0, t1=TPB, tiles=None):
                # c1 band loads on the idle GPSIMD HWDGE queue so their
                # issue never waits behind copy work on ACT/DVE
                if tiles is None:
                    c1t_a = c1_pool.tile([K0, TPB * 128], bf16, tag="c1_0")
                    c1t_b = c1_pool.tile([K1, TPB * 128], bf16, tag="c1_1")
                    tiles = [c1t_a, c1t_b]
                for k, (ks, kn) in enumerate(((0, K0), (K0, K1))):
                    nc.gpsimd.dma_start(
                        tiles[k][:, t0 * 128:t1 * 128],
                        c1_d[ks:ks + kn,
                             (band * TPB + t0) * 128:
                             (band * TPB + t1) * 128])
                return tiles

            # PE warm-up burst during the initial DMA window: ~5us of
            # back-to-back dummy matmuls flips the HAM clock gate to 2.4GHz
            # before the real stream starts.
            warm = c1_pool.tile([128, 512], bf16, tag="warm")
            nc.gpsimd.memset(warm[:], 0.0)
            for _ in range(16):
                ps_w = ps_pool.tile([128, 1024], f32, tag="ps")
                nc.tensor.matmul(ps_w[:, 0:512], warm[:, 0:128],
                                 warm[:, 0:512], start=True, stop=True)

            # band 0 c1 split: first half gates the first matmul; the
            # second half and band 1 stream in behind it
            c1_0 = load_c1(0, 0, TPB // 2)
            load_c1(0, TPB // 2, TPB, tiles=c1_0)
            c1_bufs = [c1_0, load_c1(1)]
            HT = TPB // 2             # tiles per half-band
            for band in range(BANDS):
                r0 = band * TH
                c1_sb = c1_bufs.pop(0)

                staged = st_pool.tile([128, TPB * NCOL], bf16, tag="staged")
                sap0 = staged[:]
                srow = sap0.ap[0][0]

                for half in range(2):
                    for tp in range(TPB // 4):
                        pi = half * (TPB // 4) + tp
                        ps = ps_pool.tile([128, 1024], f32, tag="ps")
                        for hf in range(2):
                            t_i = 2 * pi + hf
                            c0 = t_i * TW
                            for k, kn in enumerate((K0, K1)):
                                a1 = c1_sb[k][:]
                                lhsT = AP(a1.tensor,
                                          a1.offset + t_i * 128,
                                          [[a1.ap[0][0], kn], [1, 128]])
                                a2 = wp_sb[k][:]
                                rhs = AP(a2.tensor,
                                         a2.offset + r0 * PW + c0,
                                         [[a2.ap[0][0], kn],
                                          [PW, HH], [1, HWW]])
                                nc.tensor.matmul(
                                    ps[:, hf * 512:hf * 512 + NCOL],
                                    lhsT, rhs,
                                    start=(k == 0), stop=(k == 1))
                        # one copy moves both tiles' grams; spread over
                        # DVE / ACT / GPSIMD to keep each engine <50% busy
                        pap = ps[:]
                        src2 = AP(pap.tensor, pap.offset,
                                  [[pap.ap[0][0], 128], [512, 2],
                                   [1, NCOL]])
                        d0 = 2 * pi * NCOL
                        dst2 = staged[:, d0:d0 + 2 * NCOL]
                        if pi in (1, 8):
                            nc.vector.tensor_copy(dst2, src2)
                        else:
                            nc.scalar.copy(dst2, src2)
                        # deferred 2-ahead c1 prefetch
                        if pi == 0 and band + 2 < BANDS:
                            c1_bufs.append(load_c1(band + 2))

                    # compact half-band: per cluster of 32 consecutive
                    # partitions, window [c*64, c*64+192) of each 384-block.
                    # The very last half-band is split in two quarters so
                    # the final out-DMA drains a smaller tail.
                    win = wn_pool.tile([128, HT * WIN], bf16, tag="win")
                    wap = win[:]
                    wrow = wap.ap[0][0]
                    qsplits = ((0, HT),)
                    if band == BANDS - 1 and half == 1:
                        qsplits = ((0, 6), (6, HT))
                    for q0, q1 in qsplits:
                        for cc in range(NCLUST):
                            src = AP(sap0.tensor,
                                     sap0.offset + cc * 32 * srow
                                     + (half * HT + q0) * NCOL + cc * 64,
                                     [[srow, 32], [NCOL, q1 - q0],
                                      [1, WIN]])
                            dst = AP(wap.tensor,
                                     wap.offset + cc * 32 * wrow + q0 * WIN,
                                     [[wrow, 32], [WIN, q1 - q0],
                                      [1, WIN]])
                            nc.vector.tensor_copy(dst, src)
                        # out: one descriptor per partition.  On the SYNC
                        # queue: FIFO order defers all output traffic until
                        # the wpad input stream has drained, keeping the
                        # ring bandwidth clean for input during the
                        # data-paced phase (win bufs above absorb the lag).
                        gap = go_d[:]
                        src = AP(wap.tensor, wap.offset + q0 * WIN,
                                 [[wrow, 128], [1, (q1 - q0) * WIN]])
                        dst = AP(gap.tensor,
                                 gap.offset + (2 * band + half) * 128
                                 * (HT * WIN) + q0 * WIN,
                                 [[HT * WIN, 128], [1, (q1 - q0) * WIN]])
                        nc.sync.dma_start(dst, src)

    nc.finalize()
    _CACHE["nc"] = nc
    return nc


def kernel(c1, warped, alpha):
    import sys
    if "/opt/trn_rl_repo" not in sys.path:
        sys.path.insert(0, "/opt/trn_rl_repo")
    import ml_dtypes
    from concourse.bass_utils import run_bass_kernel_spmd

    nc = _build()
    bf = ml_dtypes.bfloat16

    in_maps = []
    for b in range(B):
        # col-padded interior only; device memsets the 4-row borders
        wpad = np.zeros((C, H, PW), np.float32)
        wpad[:, :, R:R + W] = warped[b]
        # tile c1: [C, band, a, t, b8] -> [C, band, t, c, b8, q]
        # column m = c*32 + b8*4 + q, where a = 4c + q
        c1t = np.asarray(c1[b]).reshape(C, BANDS, NCLUST, 4, TPB, TW)
        c1t = np.ascontiguousarray(c1t.transpose(0, 1, 4, 2, 5, 3))
        in_maps.append({
            "c1b": c1t.reshape(C, H * W).astype(bf),
            "wpad": wpad.reshape(C, H * PW).astype(bf),
        })

    import os
    trace = bool(int(os.environ.get("COSTVOL_TRACE", "0")))
    res = run_bass_kernel_spmd(nc, in_maps, core_ids=list(range(B)),
                               trace=trace)
    if trace:
        _CACHE["last_exec_time_ns"] = res.exec_time_ns

    # host-side: diagonal gather + mean + PReLU
    a_val = float(np.asarray(alpha).reshape(-1)[0])
    dy, dx = np.meshgrid(np.arange(9), np.arange(9), indexing="ij")
    dy = dy.reshape(-1)
    dx = dx.reshape(-1)                                      # [81]
    qq = np.arange(4)[:, None, None]
    bb8 = np.arange(TW)[None, :, None]
    # j = (q+dy)*16 + (b8+dx), shape [q, b8, 81]
    jidx = (qq + dy[None, None, :]) * HWW + bb8 + dx[None, None, :]

    out = np.empty((B, 81, H, W), np.float32)
    for b in range(B):
        g = np.asarray(res.results[b]["gout"]).astype(np.float32)
        # [band, half, c, b8, q, th, j]
        g = g.reshape(BANDS, 2, NCLUST, TW, 4, TPB // 2, WIN)
        got = np.take_along_axis(
            g, jidx.transpose(1, 0, 2)[None, None, None, :, :, None, :],
            axis=6)
        # got: [band, half, c, b8, q, th, 81] -> [81, band, c, q, half, th, b8]
        cost = got.transpose(6, 0, 2, 4, 1, 5, 3).reshape(81, H, W) * (1.0 / C)
        out[b] = np.where(cost >= 0, cost, a_val * cost)
    return out
